# revision 29
# baseline (speedup 1.0000x reference)
"""BasicBlock kernel, 1D-Winograd F(2,3) variant.

Each 3x3 conv = x-direction Winograd F(2,3) (4 planes, 2 outputs per tile)
x y-direction direct (3 dy taps):

  V(j)[c,y,tx]  = B-combos of x[c, y, 2tx+b]          (gpsimd, 4 tensor ops)
  M(j)          = sum_{dy,ci} W'(dy,j)^T @ V(j)       (PE, 24 matmuls/psum-pair)
  out[...,2tx]   = M0+M1+M2,  out[...,2tx+1] = M1-M2-M3  (DVE reduce+stt chain)

PE streams 2/3 of the direct conv's columns.  PSUM plane pairs: tileA=[M1,M2],
tileB=[M0,M3], each one bank; combinations need only one PSUM operand per op:
  rA = reduce(M1+M2); u0 = M0 + rA; d = rA - 2*M2; u1 = d - M3.
"""

import os
from contextlib import ExitStack

import numpy as np

import concourse.bass as bass
import concourse.tile as tile
from concourse import bacc, mybir
from concourse.bass_utils import run_bass_kernel_spmd

F32 = mybir.dt.float32
F16 = mybir.dt.float16

N_CORES = 8
C = 256
H = W = 32
P = 128
CB = C // P
HP = H + 2
WP = W + 2
PAD = HP * WP
TX = W // 2          # 16 winograd column pairs
NPL = 4              # planes
HALF = (H // 2) * W  # 512
NIMG = 64 // N_CORES

XR = 3
HR = 2
# plane -> (pair tile key, offset): tileA=[M1,M2], tileB=[M0,M3]; one reduce
# over tileA feeds both output chains (cheapest DVE mix).
PLANE_SLOT = {1: ("A", 0), 2: ("A", 256), 0: ("B", 0), 3: ("B", 256)}
# matmul emission / weight storage order: j=1 first (first consumed)
JORD = (1, 2, 0, 3)
JPOS = {j: q for q, j in enumerate(JORD)}


def build(nimg: int = NIMG) -> bacc.Bacc:
    nc = bacc.Bacc("TRN2", target_bir_lowering=False, debug=False, enable_asserts=True)

    x_d = nc.dram_tensor("xp", [nimg, CB, P, PAD], F16, kind="ExternalInput")
    w1_d = nc.dram_tensor("w1t", [CB, P, 3 * NPL * CB * P], F16, kind="ExternalInput")
    w2_d = nc.dram_tensor("w2t", [CB, P, 3 * NPL * CB * P], F16, kind="ExternalInput")
    bn_d = nc.dram_tensor("bnv", [P, 4 * CB], F32, kind="ExternalInput")
    y_d = nc.dram_tensor("y", [nimg, C, H, W], F32, kind="ExternalOutput")

    with tile.TileContext(nc) as tc, ExitStack() as ctx:
        wpool = ctx.enter_context(tc.tile_pool(name="weights", bufs=1))
        xpool = ctx.enter_context(tc.tile_pool(name="xpad", bufs=XR))
        vpool = ctx.enter_context(tc.tile_pool(name="vt", bufs=4))
        hpool = ctx.enter_context(tc.tile_pool(name="hpad", bufs=1))
        pspool = ctx.enter_context(tc.tile_pool(name="psum", bufs=4, space="PSUM"))
        tmppool = ctx.enter_context(tc.tile_pool(name="tmp", bufs=10))
        opool = ctx.enter_context(tc.tile_pool(name="out", bufs=3))

        xtiles, vxt, vht = {}, {}, {}

        def load_x(n, dual=False):
            t = xpool.tile([P, CB, PAD], F16, tag="xp", name=f"xt_{n}")
            for cib in range(CB):
                # dual: image 0's two halves go to different DMA queues so the
                # critical first tile lands sooner and with less variance
                eng = nc.scalar if (dual and cib == 1) else nc.sync
                eng.dma_start(t[:, cib], x_d[n, cib])
            xtiles[n] = t

        # x0 first, split across both DMA queues: image 0 gates the whole
        # pipeline (x1 is deferred until after conv1(0) is emitted, so its
        # bytes don't compete with x0/w1-q0 for HBM bandwidth).
        load_x(0, dual=True)

        # warmup tile memset on gpsimd (vector stays free for make_v(0));
        # warmup matmuls spin up the HAM clock while the x/w DMAs land.
        # 16 cold matmuls span ~6.8us, covering a full free-running HAM
        # activity window so the clock is at 2.4GHz when conv1(0) starts.
        warm = wpool.tile([P, HALF], F16, tag="warm", name="warm")
        nc.gpsimd.memset(warm[:], 0.0)
        warm_ps = pspool.tile([P, 1024], F32, tag="ps", name="warm_ps")
        n_warm = 16
        for i in range(n_warm):
            nc.tensor.matmul(
                warm_ps[:, 0:HALF], warm[:, 0:P], warm[:], start=(i == 0), stop=(i == n_warm - 1)
            )

        w1_s = [
            wpool.tile([P, 3 * NPL * CB * P], F16, tag=f"w1_{cib}", name=f"w1_{cib}")
            for cib in range(CB)
        ]
        # chunked per plane-group (j-major layout), q-major across cibs so the
        # first-consumed weights (q=0 of BOTH cibs) land first
        wchunk = 3 * CB * P
        for q in range(NPL):
            sl = slice(q * wchunk, (q + 1) * wchunk)
            for cib in range(CB):
                nc.scalar.dma_start(w1_s[cib][:, sl], w1_d[cib, :, sl])
        bn_s = wpool.tile([P, 4 * CB], F32, tag="bn", name="bn_s")
        nc.scalar.dma_start(bn_s[:], bn_d[:])
        w2_s = []
        for cib in range(CB):
            t2 = wpool.tile([P, 3 * NPL * CB * P], F16, tag=f"w2_{cib}", name=f"w2_{cib}")
            nc.scalar.dma_start(t2[:], w2_d[cib])
            w2_s.append(t2)

        def bnv(vec, cob):
            return bn_s[:, vec * CB + cob : vec * CB + cob + 1]

        hslots = [
            hpool.tile([P, CB, PAD], F16, tag=f"hp{i}", name=f"hp{i}") for i in range(HR)
        ]

        def memset_h_borders():
            for s in hslots:
                for cib in range(CB):
                    h3 = s[:, cib].rearrange("p (r c) -> p r c", c=WP)
                    nc.vector.memset(h3[:, 0 : HP : HP - 1, :], 0.0)
                    nc.vector.memset(h3[:, 1 : HP - 1, 0 : WP : WP - 1], 0.0)

        def in_tf(src, vdst, eng, eng2=None, chunk_cib=False):
            """V planes from padded source [P, CB, PAD], emitted in MM
            consumption order (j=1,2,0,3) so conv can start after op 1.
            chunk_cib: emit per-cib half ops so the cib0 half (which only
            depends on epi1's cob0 output) can run earlier; the cib1 half
            is split across both engines."""
            e2 = eng2 or eng
            s4 = src.rearrange("p b (r c) -> p b r c", c=WP)
            xb = [s4[:, :, :, b : b + 2 * TX - 1 : 2] for b in range(4)]
            v = [vdst[:, :, j] for j in range(NPL)]
            if not chunk_cib:
                eng.tensor_add(v[1], xb[1], xb[2])
                e2.tensor_sub(v[2], xb[2], xb[1])
                eng.tensor_sub(v[0], xb[0], xb[2])
                e2.tensor_sub(v[3], xb[1], xb[3])
                return
            for cib in range(CB):
                c = slice(cib, cib + 1)
                # cib1's first-consumed plane (j=1) goes to the vector engine
                # so conv2 can start after one short op; everything else stays
                # on gpsimd and hides under conv2's matmul stream.
                (e2 if cib == 1 else eng).tensor_add(
                    v[1][:, c], xb[1][:, c], xb[2][:, c])
                eng.tensor_sub(v[2][:, c], xb[2][:, c], xb[1][:, c])
                eng.tensor_sub(v[0][:, c], xb[0][:, c], xb[2][:, c])
                eng.tensor_sub(v[3][:, c], xb[1][:, c], xb[3][:, c])

        def make_v(n, store, src, eng, eng2=None, chunk_cib=False):
            vt_ = vpool.tile([P, CB, NPL, HP, TX], F16, tag="v", name=f"v_{len(store)}_{n}")
            in_tf(src, vt_, eng, eng2, chunk_cib)
            store[n] = vt_

        def conv_cob(ws, vt_, which, n, cob, halves=False):
            """24 matmuls (N=512) for one cob; returns (tileA, tileB) two-bank
            tiles, each plane a contiguous [32y x 16tx] 512-f32 region that
            stays inside one PSUM bank.
            halves: tileB is split into two [P,512] one-bank tiles (rows 0:16
            and 16:32, each holding M0|M3 halves) emitted h0-first, so the
            epilogue for h0 runs while h1's matmuls still stream (the tile
            framework tracks deps at tile granularity)."""
            pa = pspool.tile([P, 1024], F32, tag="ps", name=f"ps{which}A_{n}_{cob}")
            pb = pspool.tile([P, 1024], F32, tag="ps", name=f"ps{which}B_{n}_{cob}")
            rA = None

            def w_ap(q, dy, cib):
                return ws[cib][
                    :,
                    ((q * 3 + dy) * CB + cob) * P : ((q * 3 + dy) * CB + cob + 1) * P,
                ]

            def mm_group(j, dst, row0, rows):
                q = JPOS[j]
                for cib in range(CB):
                    for dy in range(3):
                        rhs = vt_[:, cib, j, dy + row0 : dy + row0 + rows, :]
                        nc.tensor.matmul(
                            dst,
                            w_ap(q, dy, cib),
                            rhs,
                            start=(cib == 0 and dy == 0),
                            stop=(cib == CB - 1 and dy == 2),
                            skip_group_check=halves,
                        )

            for j in JORD:
                key, off = PLANE_SLOT[j]
                dst_tile = pa if key == "A" else pb
                mm_group(j, dst_tile[:, 2 * off : 2 * off + 512], 0, H)
                if j == 2:
                    # tileA (M1, M2) complete: rA and dd (both read only
                    # tileA) hide under tileB's 12 matmuls.  u0 must NOT be
                    # hoisted: reading tileB mid-write serializes against the
                    # remaining matmuls at tile granularity.
                    rA = tmppool.tile(
                        [P, 2 * 16 * TX], F32, tag="rA", name=f"rA_{which}_{n}_{cob}"
                    )
                    nc.vector.reduce_sum(
                        rA[:],
                        pa[:].rearrange("p (j t) -> p t j", j=2),
                        axis=mybir.AxisListType.X,
                    )
                    dd = tmppool.tile(
                        [P, 2 * 16 * TX], F32, tag="dd", name=f"dd_{which}_{n}_{cob}"
                    )
                    nc.vector.scalar_tensor_tensor(
                        dd[:], pa[:, 512:1024], -2.0, rA[:],
                        op0=mybir.AluOpType.mult, op1=mybir.AluOpType.add,
                    )
            return pa, pb, rA, dd

        def combine(n, which, pb, rA, dd, cob, fs=None, tag=""):
            """u0=M0+rA and u1=dd-M3 after the matmuls (rA, dd were emitted
            inside conv_cob, hidden under tileB's matmuls)."""
            fs = fs or slice(0, 512)
            ln = fs.stop - fs.start
            u0 = tmppool.tile([P, ln], F32, tag="u0", name=f"u0_{which}_{n}_{cob}_{tag}")
            nc.vector.scalar_tensor_tensor(
                u0[:], pb[:, fs], 1.0, rA[:, fs],
                op0=mybir.AluOpType.mult, op1=mybir.AluOpType.add,
            )
            u1 = tmppool.tile([P, ln], F32, tag="u1", name=f"u1_{which}_{n}_{cob}_{tag}")
            nc.vector.scalar_tensor_tensor(
                u1[:], pb[:, 512 + fs.start : 512 + fs.stop], -1.0, dd[:, fs],
                op0=mybir.AluOpType.mult, op1=mybir.AluOpType.add,
            )
            return u0, u1

        def epi1_cob(n, cob, pb, rA, ddv):
            hdst = hslots[n % HR]
            h3 = hdst[:, cob].rearrange("p (r c) -> p r c", c=WP)
            u0, u1 = combine(n, 1, pb, rA, ddv, cob)
            for u, t in ((0, u0), (1, u1)):
                uv = t.rearrange("p (r q) -> p r q", q=TX)
                nc.scalar.activation(
                    h3[:, 1 : H + 1, 1 + u : 1 + u + 2 * TX - 1 : 2],
                    uv[:],
                    mybir.ActivationFunctionType.Relu,
                    bias=bnv(1, cob),
                    scale=bnv(0, cob),
                )

        def epi2_cob(n, cob, pb, rA, ddv, halves=False):
            xsrc = xtiles[n]
            ot = opool.tile([P, H * W], F32, tag="ot", name=f"ot_{n}_{cob}")
            ov = ot.rearrange("p (r c) -> p r c", c=W)
            x3 = xsrc[:, cob].rearrange("p (r c) -> p r c", c=WP)
            y3 = y_d[n, cob * P : (cob + 1) * P].rearrange("c h w -> c (h w)")
            # u0/u1 at full width (2 big DVE ops beat 4 small ones); for the
            # tail cob the rr/ACT/DMA chain is row-halved so the first output
            # DMA issues earlier.
            u0, u1 = combine(n, 2, pb, rA, ddv, cob)
            nh = 2 if halves else 1
            rh = H // nh
            for hf in range(nh):
                rs = slice(hf * rh, (hf + 1) * rh)
                for u, t in ((0, u0), (1, u1)):
                    uv = t.rearrange("p (r q) -> p r q", q=TX)[:, rs]
                    rr = tmppool.tile(
                        [P, rh * TX], F32, tag="rr", name=f"rr_{n}_{cob}_{u}_{hf}"
                    )
                    rv = rr.rearrange("p (r q) -> p r q", q=TX)
                    nc.vector.scalar_tensor_tensor(
                        rv[:],
                        uv[:],
                        bnv(2, cob),
                        x3[:, 1 + hf * rh : 1 + (hf + 1) * rh,
                           1 + u : 1 + u + 2 * TX - 1 : 2],
                        op0=mybir.AluOpType.mult,
                        op1=mybir.AluOpType.add,
                    )
                    nc.scalar.activation(
                        ov[:, rs, u : u + 2 * TX - 1 : 2],
                        rv[:],
                        mybir.ActivationFunctionType.Relu,
                        bias=bnv(3, cob),
                        scale=1.0,
                    )
                if halves:
                    nc.sync.dma_start(y3[:, hf * HALF : (hf + 1) * HALF],
                                      ot[:, hf * HALF : (hf + 1) * HALF])
            if not halves:
                for half in range(2):
                    nc.sync.dma_start(
                        y3[:, half * HALF : (half + 1) * HALF],
                        ot[:, half * HALF : (half + 1) * HALF],
                    )

        # ---- pipeline ----
        # engine program orders:
        #   PE:  conv1(0), conv1(1), conv2(0), conv1(2), conv2(1), ...
        #   DVE: epi1(0), epi1(1), epi2(0), epi1(2), epi2(1), ...
        # epi1(n+1) is emitted before conv2(n) so the PSUM slots conv2(n)
        # waits on are released by vector-engine work that is ahead of it.
        def conv1_and_epi1(n):
            for cob in range(CB):
                pa, pb, rA, ddv = conv_cob(w1_s, vxt[n], 1, n, cob)
                epi1_cob(n, cob, pb, rA, ddv)
            vxt.pop(n)

        def conv2_and_epi2(n):
            for cob in range(CB):
                # last cob of the last image: half-granularity epilogue so the
                # first output DMA issues ~2us earlier (shorter drain tail)
                hv = n == nimg - 1 and cob == CB - 1
                pa, pb, rA, ddv = conv_cob(w2_s, vht[n], 2, n, cob, halves=hv)
                epi2_cob(n, cob, pb, rA, ddv, halves=hv)
            vht.pop(n)
            del xtiles[n]

        make_v(0, vxt, xtiles[0], nc.vector, nc.gpsimd)  # split across two engines
        memset_h_borders()
        conv1_and_epi1(0)
        if nimg > 1:
            load_x(1)
        for n in range(nimg):
            if n + 1 < nimg:
                make_v(n + 1, vxt, xtiles[n + 1], nc.gpsimd)
            if n < nimg - 1 or nimg == 1:
                make_v(n, vht, hslots[n % HR], nc.gpsimd, None,
                       chunk_cib=(nimg == 1))
            if n + 1 < nimg:
                conv1_and_epi1(n + 1)
                if n + 1 == nimg - 1:
                    # last image's h transform, hoisted ahead of conv2(n) so
                    # its ops aren't queued behind epi2(n)'s psum-gated stts.
                    # Per-cib chunks: cib0 can start right after epi1(cob0);
                    # cib1's first-consumed plane goes to the vector engine.
                    make_v(n + 1, vht, hslots[(n + 1) % HR], nc.gpsimd,
                           nc.vector, chunk_cib=True)
            conv2_and_epi2(n)
            if n + 2 < nimg:
                load_x(n + 2)

    nc.compile()
    return nc


_NC_CACHE: dict = {}


def _get_nc(nimg: int = NIMG):
    if nimg not in _NC_CACHE:
        _NC_CACHE[nimg] = build(nimg)
    return _NC_CACHE[nimg]


_G = np.array(
    [[1, 0, 0], [0.5, 0.5, 0.5], [0.5, -0.5, 0.5], [0, 0, 1]], np.float32
)


def _prep_host(w1, g1, b1, rm1, rv1, w2, g2, b2, rm2, rv2):
    eps = 1e-5
    f = np.float32
    inv1 = (np.asarray(g1, f) / np.sqrt(np.asarray(rv1, f) + eps)).astype(f)
    b1p = (np.asarray(b1, f) - np.asarray(rm1, f) * inv1).astype(f)
    inv2 = (np.asarray(g2, f) / np.sqrt(np.asarray(rv2, f) + eps)).astype(f)
    b2p = (np.asarray(b2, f) - np.asarray(rm2, f) * inv2).astype(f)
    bnv = np.zeros((P, 4 * CB), f)
    for vi, v in enumerate([inv1, b1p, inv2, b2p]):
        for cob in range(CB):
            bnv[:, vi * CB + cob] = v[cob * P : (cob + 1) * P]

    def wt(w):
        w = np.asarray(w, f)
        wp = np.einsum("oidk,jk->oidj", w, _G)          # [o, i, dy, j]
        wp = wp.reshape(CB, P, CB, P, 3, NPL)            # [cob, co, cib, ci, dy, j]
        wp = wp[..., list(JORD)]                         # planes in consumption order
        wp = wp.transpose(2, 3, 5, 4, 0, 1)              # [cib, ci, q, dy, cob, co]
        return np.ascontiguousarray(
            wp.reshape(CB, P, 3 * NPL * CB * P).astype(np.float16)
        )

    return wt(w1), wt(w2), bnv


def _pad_x(x):
    n = x.shape[0]
    xp = np.zeros((n, C, HP, WP), np.float32)
    xp[:, :, 1 : H + 1, 1 : W + 1] = x
    return np.ascontiguousarray(xp.reshape(n, CB, P, PAD).astype(np.float16))


def make_in_maps(x, w1, g1, b1, rm1, rv1, w2, g2, b2, rm2, rv2):
    x = np.asarray(x, np.float32)
    nimg = x.shape[0] // N_CORES
    w1t, w2t, bnv = _prep_host(w1, g1, b1, rm1, rv1, w2, g2, b2, rm2, rv2)
    return [
        {
            "xp": _pad_x(x[c * nimg : (c + 1) * nimg]),
            "w1t": w1t,
            "w2t": w2t,
            "bnv": bnv,
        }
        for c in range(N_CORES)
    ]


def kernel(x, w1, g1, b1, rm1, rv1, w2, g2, b2, rm2, rv2):
    x = np.asarray(x, np.float32)
    assert x.shape[0] % N_CORES == 0
    nc = _get_nc(x.shape[0] // N_CORES)
    in_maps = make_in_maps(x, w1, g1, b1, rm1, rv1, w2, g2, b2, rm2, rv2)
    res = run_bass_kernel_spmd(nc, in_maps, list(range(N_CORES)))
    return np.ascontiguousarray(
        np.concatenate([res.results[c]["y"] for c in range(N_CORES)], axis=0)
    )



# revision 30
# speedup vs baseline: 1.1930x; 1.1930x over previous
"""BasicBlock kernel, 1D-Winograd F(2,3) variant.

Each 3x3 conv = x-direction Winograd F(2,3) (4 planes, 2 outputs per tile)
x y-direction direct (3 dy taps):

  V(j)[c,y,tx]  = B-combos of x[c, y, 2tx+b]          (gpsimd, 4 tensor ops)
  M(j)          = sum_{dy,ci} W'(dy,j)^T @ V(j)       (PE, 24 matmuls/psum-pair)
  out[...,2tx]   = M0+M1+M2,  out[...,2tx+1] = M1-M2-M3  (DVE reduce+stt chain)

PE streams 2/3 of the direct conv's columns.  PSUM plane pairs: tileA=[M1,M2],
tileB=[M0,M3], each one bank; combinations need only one PSUM operand per op:
  rA = reduce(M1+M2); u0 = M0 + rA; d = rA - 2*M2; u1 = d - M3.
"""

import os
from contextlib import ExitStack

import numpy as np

import concourse.bass as bass
import concourse.tile as tile
from concourse import bacc, mybir
from concourse.bass_utils import run_bass_kernel_spmd

F32 = mybir.dt.float32
F16 = mybir.dt.float16

N_CORES = 8
C = 256
H = W = 32
P = 128
CB = C // P
HP = H + 2
WP = W + 2
PAD = HP * WP
TX = W // 2          # 16 winograd column pairs
NPL = 4              # planes
HALF = (H // 2) * W  # 512
NIMG = 64 // N_CORES

XR = 3
HR = 2
# plane -> (pair tile key, offset): tileA=[M1,M2], tileB=[M0,M3]; one reduce
# over tileA feeds both output chains (cheapest DVE mix).
PLANE_SLOT = {1: ("A", 0), 2: ("A", 256), 0: ("B", 0), 3: ("B", 256)}
# matmul emission / weight storage order: j=1 first (first consumed)
JORD = (1, 2, 0, 3)
JPOS = {j: q for q, j in enumerate(JORD)}


def build(nimg: int = NIMG) -> bacc.Bacc:
    nc = bacc.Bacc("TRN2", target_bir_lowering=False, debug=False, enable_asserts=True)

    x_d = nc.dram_tensor("xp", [nimg, CB, P, PAD], F16, kind="ExternalInput")
    w1_d = nc.dram_tensor("w1t", [CB, P, 3 * NPL * CB * P], F16, kind="ExternalInput")
    w2_d = nc.dram_tensor("w2t", [CB, P, 3 * NPL * CB * P], F16, kind="ExternalInput")
    bn_d = nc.dram_tensor("bnv", [P, 4 * CB], F32, kind="ExternalInput")
    y_d = nc.dram_tensor("y", [nimg, C, H, W], F32, kind="ExternalOutput")

    with tile.TileContext(nc) as tc, ExitStack() as ctx:
        wpool = ctx.enter_context(tc.tile_pool(name="weights", bufs=1))
        xpool = ctx.enter_context(tc.tile_pool(name="xpad", bufs=XR))
        vpool = ctx.enter_context(tc.tile_pool(name="vt", bufs=4))
        hpool = ctx.enter_context(tc.tile_pool(name="hpad", bufs=1))
        pspool = ctx.enter_context(tc.tile_pool(name="psum", bufs=4, space="PSUM"))
        tmppool = ctx.enter_context(tc.tile_pool(name="tmp", bufs=10))
        opool = ctx.enter_context(tc.tile_pool(name="out", bufs=3))

        xtiles, vxt, vht = {}, {}, {}

        def load_x(n, dual=False):
            t = xpool.tile([P, CB, PAD], F16, tag="xp", name=f"xt_{n}")
            for cib in range(CB):
                # dual: image 0's two halves go to different DMA queues so the
                # critical first tile lands sooner and with less variance
                eng = nc.scalar if (dual and cib == 1) else nc.sync
                eng.dma_start(t[:, cib], x_d[n, cib])
            xtiles[n] = t

        # x0 first, split across both DMA queues: image 0 gates the whole
        # pipeline (x1 is deferred until after conv1(0) is emitted, so its
        # bytes don't compete with x0/w1-q0 for HBM bandwidth).
        load_x(0, dual=True)

        # warmup tile memset on gpsimd (vector stays free for make_v(0));
        # warmup matmuls spin up the HAM clock while the x/w DMAs land.
        # 16 cold matmuls span ~6.8us, covering a full free-running HAM
        # activity window so the clock is at 2.4GHz when conv1(0) starts.
        warm = wpool.tile([P, HALF], F16, tag="warm", name="warm")
        nc.gpsimd.memset(warm[:], 0.0)
        warm_ps = pspool.tile([P, 1024], F32, tag="ps", name="warm_ps")
        n_warm = 16
        for i in range(n_warm):
            nc.tensor.matmul(
                warm_ps[:, 0:HALF], warm[:, 0:P], warm[:], start=(i == 0), stop=(i == n_warm - 1)
            )

        w1_s = [
            wpool.tile([P, 3 * NPL * CB * P], F16, tag=f"w1_{cib}", name=f"w1_{cib}")
            for cib in range(CB)
        ]
        # two chunks per cib (plane-groups q0+q1 / q2+q3, j-major layout),
        # chunk-major across cibs so the first-consumed weights land first;
        # fewer DMA issues = less queue serialization at startup
        wchunk = 2 * 3 * CB * P
        for q in range(NPL // 2):
            sl = slice(q * wchunk, (q + 1) * wchunk)
            for cib in range(CB):
                nc.scalar.dma_start(w1_s[cib][:, sl], w1_d[cib, :, sl])
        bn_s = wpool.tile([P, 4 * CB], F32, tag="bn", name="bn_s")
        nc.scalar.dma_start(bn_s[:], bn_d[:])
        w2_s = []
        for cib in range(CB):
            t2 = wpool.tile([P, 3 * NPL * CB * P], F16, tag=f"w2_{cib}", name=f"w2_{cib}")
            nc.scalar.dma_start(t2[:], w2_d[cib])
            w2_s.append(t2)

        def bnv(vec, cob):
            return bn_s[:, vec * CB + cob : vec * CB + cob + 1]

        hslots = [
            hpool.tile([P, CB, PAD], F16, tag=f"hp{i}", name=f"hp{i}") for i in range(HR)
        ]

        def memset_h_borders():
            for s in hslots:
                for cib in range(CB):
                    h3 = s[:, cib].rearrange("p (r c) -> p r c", c=WP)
                    nc.vector.memset(h3[:, 0 : HP : HP - 1, :], 0.0)
                    nc.vector.memset(h3[:, 1 : HP - 1, 0 : WP : WP - 1], 0.0)

        def in_tf(src, vdst, eng, eng2=None, chunk_cib=False):
            """V planes from padded source [P, CB, PAD], emitted in MM
            consumption order (j=1,2,0,3) so conv can start after op 1.
            chunk_cib: emit per-cib half ops so the cib0 half (which only
            depends on epi1's cob0 output) can run earlier; the cib1 half
            is split across both engines."""
            e2 = eng2 or eng
            s4 = src.rearrange("p b (r c) -> p b r c", c=WP)
            xb = [s4[:, :, :, b : b + 2 * TX - 1 : 2] for b in range(4)]
            v = [vdst[:, :, j] for j in range(NPL)]
            if not chunk_cib:
                eng.tensor_add(v[1], xb[1], xb[2])
                e2.tensor_sub(v[2], xb[2], xb[1])
                eng.tensor_sub(v[0], xb[0], xb[2])
                e2.tensor_sub(v[3], xb[1], xb[3])
                return
            for cib in range(CB):
                c = slice(cib, cib + 1)
                # cib1's first-consumed plane (j=1) goes to the vector engine
                # so conv2 can start after one short op; everything else stays
                # on gpsimd and hides under conv2's matmul stream.
                (e2 if cib == 1 else eng).tensor_add(
                    v[1][:, c], xb[1][:, c], xb[2][:, c])
                eng.tensor_sub(v[2][:, c], xb[2][:, c], xb[1][:, c])
                eng.tensor_sub(v[0][:, c], xb[0][:, c], xb[2][:, c])
                eng.tensor_sub(v[3][:, c], xb[1][:, c], xb[3][:, c])

        def make_v(n, store, src, eng, eng2=None, chunk_cib=False):
            vt_ = vpool.tile([P, CB, NPL, HP, TX], F16, tag="v", name=f"v_{len(store)}_{n}")
            in_tf(src, vt_, eng, eng2, chunk_cib)
            store[n] = vt_

        def conv_cob(ws, vt_, which, n, cob, halves=False):
            """24 matmuls (N=512) for one cob; returns (tileA, tileB) two-bank
            tiles, each plane a contiguous [32y x 16tx] 512-f32 region that
            stays inside one PSUM bank.
            halves: tileB is split into two [P,512] one-bank tiles (rows 0:16
            and 16:32, each holding M0|M3 halves) emitted h0-first, so the
            epilogue for h0 runs while h1's matmuls still stream (the tile
            framework tracks deps at tile granularity)."""
            pa = pspool.tile([P, 1024], F32, tag="ps", name=f"ps{which}A_{n}_{cob}")
            pb = pspool.tile([P, 1024], F32, tag="ps", name=f"ps{which}B_{n}_{cob}")
            rA = None

            def w_ap(q, dy, cib):
                return ws[cib][
                    :,
                    ((q * 3 + dy) * CB + cob) * P : ((q * 3 + dy) * CB + cob + 1) * P,
                ]

            def mm_group(j, dst, row0, rows):
                q = JPOS[j]
                for cib in range(CB):
                    for dy in range(3):
                        rhs = vt_[:, cib, j, dy + row0 : dy + row0 + rows, :]
                        nc.tensor.matmul(
                            dst,
                            w_ap(q, dy, cib),
                            rhs,
                            start=(cib == 0 and dy == 0),
                            stop=(cib == CB - 1 and dy == 2),
                            skip_group_check=halves,
                        )

            for j in JORD:
                key, off = PLANE_SLOT[j]
                dst_tile = pa if key == "A" else pb
                mm_group(j, dst_tile[:, 2 * off : 2 * off + 512], 0, H)
                if j == 2:
                    # tileA (M1, M2) complete: rA and dd (both read only
                    # tileA) hide under tileB's 12 matmuls.  u0 must NOT be
                    # hoisted: reading tileB mid-write serializes against the
                    # remaining matmuls at tile granularity.
                    rA = tmppool.tile(
                        [P, 2 * 16 * TX], F32, tag="rA", name=f"rA_{which}_{n}_{cob}"
                    )
                    nc.vector.reduce_sum(
                        rA[:],
                        pa[:].rearrange("p (j t) -> p t j", j=2),
                        axis=mybir.AxisListType.X,
                    )
                    dd = tmppool.tile(
                        [P, 2 * 16 * TX], F32, tag="dd", name=f"dd_{which}_{n}_{cob}"
                    )
                    nc.vector.scalar_tensor_tensor(
                        dd[:], pa[:, 512:1024], -2.0, rA[:],
                        op0=mybir.AluOpType.mult, op1=mybir.AluOpType.add,
                    )
            return pa, pb, rA, dd

        def combine(n, which, pb, rA, dd, cob, fs=None, tag=""):
            """u0=M0+rA and u1=dd-M3 after the matmuls (rA, dd were emitted
            inside conv_cob, hidden under tileB's matmuls)."""
            fs = fs or slice(0, 512)
            ln = fs.stop - fs.start
            u0 = tmppool.tile([P, ln], F32, tag="u0", name=f"u0_{which}_{n}_{cob}_{tag}")
            nc.vector.scalar_tensor_tensor(
                u0[:], pb[:, fs], 1.0, rA[:, fs],
                op0=mybir.AluOpType.mult, op1=mybir.AluOpType.add,
            )
            u1 = tmppool.tile([P, ln], F32, tag="u1", name=f"u1_{which}_{n}_{cob}_{tag}")
            nc.vector.scalar_tensor_tensor(
                u1[:], pb[:, 512 + fs.start : 512 + fs.stop], -1.0, dd[:, fs],
                op0=mybir.AluOpType.mult, op1=mybir.AluOpType.add,
            )
            return u0, u1

        def epi1_cob(n, cob, pb, rA, ddv):
            hdst = hslots[n % HR]
            h3 = hdst[:, cob].rearrange("p (r c) -> p r c", c=WP)
            u0, u1 = combine(n, 1, pb, rA, ddv, cob)
            for u, t in ((0, u0), (1, u1)):
                uv = t.rearrange("p (r q) -> p r q", q=TX)
                nc.scalar.activation(
                    h3[:, 1 : H + 1, 1 + u : 1 + u + 2 * TX - 1 : 2],
                    uv[:],
                    mybir.ActivationFunctionType.Relu,
                    bias=bnv(1, cob),
                    scale=bnv(0, cob),
                )

        def epi2_cob(n, cob, pb, rA, ddv, halves=False):
            xsrc = xtiles[n]
            ot = opool.tile([P, H * W], F32, tag="ot", name=f"ot_{n}_{cob}")
            ov = ot.rearrange("p (r c) -> p r c", c=W)
            x3 = xsrc[:, cob].rearrange("p (r c) -> p r c", c=WP)
            y3 = y_d[n, cob * P : (cob + 1) * P].rearrange("c h w -> c (h w)")
            # u0/u1 at full width (2 big DVE ops beat 4 small ones); for the
            # tail cob the rr/ACT/DMA chain is row-halved so the first output
            # DMA issues earlier.
            u0, u1 = combine(n, 2, pb, rA, ddv, cob)
            nh = 2 if halves else 1
            rh = H // nh
            for hf in range(nh):
                rs = slice(hf * rh, (hf + 1) * rh)
                for u, t in ((0, u0), (1, u1)):
                    uv = t.rearrange("p (r q) -> p r q", q=TX)[:, rs]
                    rr = tmppool.tile(
                        [P, rh * TX], F32, tag="rr", name=f"rr_{n}_{cob}_{u}_{hf}"
                    )
                    rv = rr.rearrange("p (r q) -> p r q", q=TX)
                    nc.vector.scalar_tensor_tensor(
                        rv[:],
                        uv[:],
                        bnv(2, cob),
                        x3[:, 1 + hf * rh : 1 + (hf + 1) * rh,
                           1 + u : 1 + u + 2 * TX - 1 : 2],
                        op0=mybir.AluOpType.mult,
                        op1=mybir.AluOpType.add,
                    )
                    nc.scalar.activation(
                        ov[:, rs, u : u + 2 * TX - 1 : 2],
                        rv[:],
                        mybir.ActivationFunctionType.Relu,
                        bias=bnv(3, cob),
                        scale=1.0,
                    )
                if halves:
                    nc.sync.dma_start(y3[:, hf * HALF : (hf + 1) * HALF],
                                      ot[:, hf * HALF : (hf + 1) * HALF])
            if not halves:
                for half in range(2):
                    nc.sync.dma_start(
                        y3[:, half * HALF : (half + 1) * HALF],
                        ot[:, half * HALF : (half + 1) * HALF],
                    )

        # ---- pipeline ----
        # engine program orders:
        #   PE:  conv1(0), conv1(1), conv2(0), conv1(2), conv2(1), ...
        #   DVE: epi1(0), epi1(1), epi2(0), epi1(2), epi2(1), ...
        # epi1(n+1) is emitted before conv2(n) so the PSUM slots conv2(n)
        # waits on are released by vector-engine work that is ahead of it.
        def conv1_and_epi1(n):
            for cob in range(CB):
                pa, pb, rA, ddv = conv_cob(w1_s, vxt[n], 1, n, cob)
                epi1_cob(n, cob, pb, rA, ddv)
            vxt.pop(n)

        def conv2_and_epi2(n):
            for cob in range(CB):
                # last cob of the last image: half-granularity epilogue so the
                # first output DMA issues ~2us earlier (shorter drain tail)
                hv = n == nimg - 1 and cob == CB - 1
                pa, pb, rA, ddv = conv_cob(w2_s, vht[n], 2, n, cob, halves=hv)
                epi2_cob(n, cob, pb, rA, ddv, halves=hv)
            vht.pop(n)
            del xtiles[n]

        make_v(0, vxt, xtiles[0], nc.vector, nc.gpsimd)  # split across two engines
        memset_h_borders()
        conv1_and_epi1(0)
        if nimg > 1:
            load_x(1)
        for n in range(nimg):
            if n + 1 < nimg:
                make_v(n + 1, vxt, xtiles[n + 1], nc.gpsimd)
            if n < nimg - 1 or nimg == 1:
                make_v(n, vht, hslots[n % HR], nc.gpsimd, None,
                       chunk_cib=(nimg == 1))
            if n + 1 < nimg:
                conv1_and_epi1(n + 1)
                if n + 1 == nimg - 1:
                    # last image's h transform, hoisted ahead of conv2(n) so
                    # its ops aren't queued behind epi2(n)'s psum-gated stts.
                    # Per-cib chunks: cib0 can start right after epi1(cob0);
                    # cib1's first-consumed plane goes to the vector engine.
                    make_v(n + 1, vht, hslots[(n + 1) % HR], nc.gpsimd,
                           nc.vector, chunk_cib=True)
            conv2_and_epi2(n)
            if n + 2 < nimg:
                load_x(n + 2)

    nc.compile()
    return nc


_NC_CACHE: dict = {}


def _get_nc(nimg: int = NIMG):
    if nimg not in _NC_CACHE:
        _NC_CACHE[nimg] = build(nimg)
    return _NC_CACHE[nimg]


_G = np.array(
    [[1, 0, 0], [0.5, 0.5, 0.5], [0.5, -0.5, 0.5], [0, 0, 1]], np.float32
)


def _prep_host(w1, g1, b1, rm1, rv1, w2, g2, b2, rm2, rv2):
    eps = 1e-5
    f = np.float32
    inv1 = (np.asarray(g1, f) / np.sqrt(np.asarray(rv1, f) + eps)).astype(f)
    b1p = (np.asarray(b1, f) - np.asarray(rm1, f) * inv1).astype(f)
    inv2 = (np.asarray(g2, f) / np.sqrt(np.asarray(rv2, f) + eps)).astype(f)
    b2p = (np.asarray(b2, f) - np.asarray(rm2, f) * inv2).astype(f)
    bnv = np.zeros((P, 4 * CB), f)
    for vi, v in enumerate([inv1, b1p, inv2, b2p]):
        for cob in range(CB):
            bnv[:, vi * CB + cob] = v[cob * P : (cob + 1) * P]

    def wt(w):
        w = np.asarray(w, f)
        wp = np.einsum("oidk,jk->oidj", w, _G)          # [o, i, dy, j]
        wp = wp.reshape(CB, P, CB, P, 3, NPL)            # [cob, co, cib, ci, dy, j]
        wp = wp[..., list(JORD)]                         # planes in consumption order
        wp = wp.transpose(2, 3, 5, 4, 0, 1)              # [cib, ci, q, dy, cob, co]
        return np.ascontiguousarray(
            wp.reshape(CB, P, 3 * NPL * CB * P).astype(np.float16)
        )

    return wt(w1), wt(w2), bnv


def _pad_x(x):
    n = x.shape[0]
    xp = np.zeros((n, C, HP, WP), np.float32)
    xp[:, :, 1 : H + 1, 1 : W + 1] = x
    return np.ascontiguousarray(xp.reshape(n, CB, P, PAD).astype(np.float16))


def make_in_maps(x, w1, g1, b1, rm1, rv1, w2, g2, b2, rm2, rv2):
    x = np.asarray(x, np.float32)
    nimg = x.shape[0] // N_CORES
    w1t, w2t, bnv = _prep_host(w1, g1, b1, rm1, rv1, w2, g2, b2, rm2, rv2)
    return [
        {
            "xp": _pad_x(x[c * nimg : (c + 1) * nimg]),
            "w1t": w1t,
            "w2t": w2t,
            "bnv": bnv,
        }
        for c in range(N_CORES)
    ]


def kernel(x, w1, g1, b1, rm1, rv1, w2, g2, b2, rm2, rv2):
    x = np.asarray(x, np.float32)
    assert x.shape[0] % N_CORES == 0
    nc = _get_nc(x.shape[0] // N_CORES)
    in_maps = make_in_maps(x, w1, g1, b1, rm1, rv1, w2, g2, b2, rm2, rv2)
    res = run_bass_kernel_spmd(nc, in_maps, list(range(N_CORES)))
    return np.ascontiguousarray(
        np.concatenate([res.results[c]["y"] for c in range(N_CORES)], axis=0)
    )

